# revision 20
# baseline (speedup 1.0000x reference)
"""Trainium2 Bass kernel for BotNet-style attention (4 heads, 64x64 map,
dh=128, decomposed 2D relative position bias).

Sharding: 8 cores = 4 heads x 2 query-halves. Each core computes its head's
q/k/v from the full fmap, builds the rel-pos bias row tensors on chip, and
runs attention in "transposed sim" orientation (keys on partitions, queries
on free dim):

  simT[k, q] = K^T.T @ Q^T  (+ bias via indicator-matmul accumulation)
  expT = exp(SCALE * simT - 4)              (ACT, PSUM->SBUF fp16)

The output matmul is flipped (expT chunk as stationary, V as moving) so the
result lands as [q, d] blocks, and V carries an extra all-ones column so the
softmax denominator accumulates in the same PSUM tile for free:

  out[q, 0:128 | 128] = sum_k expT[k, q-blk]^T @ [V[k, :] | 1]

Normalization is then a per-partition reciprocal + tensor_scalar multiply on
DVE (no cross-partition reduce, no broadcast matmul, no accumulate chain).

V is built directly in [key, d] orientation via matmul(F_chunk, W_v) with the
fmap chunk as the stationary operand, so no PE transposes are needed.

The rel-pos bias decomposes per query q=(hq,wq), key k=(hk,wk) as
  bias = Rh[q, hk-hq+63] + Rw[q, wk-wq+63]
The row tensors BT are built from per-window matmuls writing disjoint PSUM
bands and fold into sim via one accumulating matmul against a 0/1 indicator
matrix per key chunk.

Per-core inputs are key-permuted (own query half first) so the SPMD graph is
identical across cores; all per-core differences live in the input data.
"""

import numpy as np
import ml_dtypes

C, H, W = 512, 64, 64
HEADS, DH = 4, 128
L = H * W           # 4096
NQ = L // 2         # 2048 queries per core
QB = 1024           # query block
SCALE = DH ** -0.5
NCORES = 8

_GRAPH = None


def _build_graph():
    from concourse import bacc
    import concourse.mybir as mybir
    import concourse.tile as tile

    f32 = mybir.dt.float32
    bf16 = mybir.dt.bfloat16
    fp16 = mybir.dt.float16
    EXPF = mybir.ActivationFunctionType.Exp

    nc = bacc.Bacc(None)

    fmap_p = nc.declare_dram_parameter("fmapc", [16 * 128, 1024], bf16, isOutput=False)
    wt_p = nc.declare_dram_parameter("wt", [C, 384], bf16, isOutput=False)
    relh_p = nc.declare_dram_parameter("relh", [128, 96], bf16, isOutput=False)
    relw_p = nc.declare_dram_parameter("relw", [128, 127], bf16, isOutput=False)
    ind_p = nc.declare_dram_parameter("ind", [4 * 128, 1024], bf16, isOutput=False)
    bias4_p = nc.declare_dram_parameter("bias4", [128, 1], f32, isOutput=False)
    out_p = nc.declare_dram_parameter("out", [128, NQ], fp16, isOutput=True)

    with tile.TileContext(nc) as tc:
        with tc.tile_pool(name="const", bufs=1) as cpool, \
             tc.tile_pool(name="big", bufs=1) as big, \
             tc.tile_pool(name="work", bufs=2) as work:

            # warm tile memset first in the gpsimd stream so PE warmup
            # matmuls can start right after the init barrier
            warm_sb = work.tile([128, 512], bf16, name="warm_sb", tag="warm")
            nc.gpsimd.memset(warm_sb, 0.0)

            relh_sb = cpool.tile([128, 96], bf16, name="relh_sb")
            relw_sb = cpool.tile([128, 127], bf16, name="relw_sb")
            ind_sb = cpool.tile([128, L], bf16, name="ind_sb")
            bias4_sb = cpool.tile([128, 1], f32, name="bias4_sb")

            F4 = [big.tile([128, L], bf16, name=f"F{c}") for c in range(4)]
            W4 = []
            for c in range(4):
                W4.append(big.tile([128, 384], bf16, name=f"W{c}"))

            # V with a ones column per 129-wide key-chunk slab: cols
            # [j*129, j*129+128) = V rows, col j*129+128 = 1.0.  One tile per
            # 4-chunk group so a consumer only waits on its own group's evac
            # (tile-granular deps) -- required for mid-loop V build fillers.
            Vng = [big.tile([128, 4 * 129], fp16, name=f"Vn{g}") for g in range(8)]
            nc.gpsimd.memset(Vng[0], 1.0)
            nc.gpsimd.memset(Vng[1], 1.0)

            QT = big.tile([128, NQ], bf16, name="QT")
            KTt = [big.tile([128, 1024], bf16, name=f"KT{t}") for t in range(4)]
            BT = big.tile([128, NQ], bf16, name="BT")

            # ---- DMA: weights first (unblock qkv matmuls), fmap t-major
            # striped across three engine queues, ind last ----
            def fblk(c, t):
                b = c * 4 + t
                return fmap_p[b * 128:(b + 1) * 128, :]

            nc.sync.dma_start(out=W4[0], in_=wt_p[0:128, :])
            nc.scalar.dma_start(out=W4[1], in_=wt_p[128:256, :])
            nc.scalar.dma_start(out=W4[2], in_=wt_p[256:384, :])
            nc.sync.dma_start(out=W4[3], in_=wt_p[384:512, :])
            # tiny consts ride the hw queues ahead of fmap (SWDGE is far too
            # slow for anything the bias phase waits on)
            nc.sync.dma_start(out=relh_sb, in_=relh_p[:, :])
            nc.scalar.dma_start(out=relw_sb, in_=relw_p[:, :])
            nc.sync.dma_start(out=bias4_sb, in_=bias4_p[:, :])
            # fmap: c0+c3 on sync, c1+c2 on scalar (only SP/ACT have hw DGE)
            for h in range(2):
                nc.sync.dma_start(out=F4[0][:, h * 512:(h + 1) * 512],
                                  in_=fblk(0, 0)[:, h * 512:(h + 1) * 512])
                nc.scalar.dma_start(out=F4[1][:, h * 512:(h + 1) * 512],
                                    in_=fblk(1, 0)[:, h * 512:(h + 1) * 512])
                nc.sync.dma_start(out=F4[3][:, h * 512:(h + 1) * 512],
                                  in_=fblk(3, 0)[:, h * 512:(h + 1) * 512])
                nc.scalar.dma_start(out=F4[2][:, h * 512:(h + 1) * 512],
                                    in_=fblk(2, 0)[:, h * 512:(h + 1) * 512])
            for t in range(1, 3):
                nc.sync.dma_start(out=F4[0][:, t * 1024:(t + 1) * 1024], in_=fblk(0, t))
                nc.scalar.dma_start(out=F4[1][:, t * 1024:(t + 1) * 1024], in_=fblk(1, t))
                nc.sync.dma_start(out=F4[3][:, t * 1024:(t + 1) * 1024], in_=fblk(3, t))
                nc.scalar.dma_start(out=F4[2][:, t * 1024:(t + 1) * 1024], in_=fblk(2, t))
            for k in range(4):
                (nc.sync if k % 2 == 0 else nc.scalar).dma_start(
                    out=ind_sb[:, k * 1024:(k + 1) * 1024],
                    in_=ind_p[k * 128:(k + 1) * 128, :])
            nc.sync.dma_start(out=F4[0][:, 3 * 1024:4 * 1024], in_=fblk(0, 3))
            nc.scalar.dma_start(out=F4[1][:, 3 * 1024:4 * 1024], in_=fblk(1, 3))
            nc.sync.dma_start(out=F4[3][:, 3 * 1024:4 * 1024], in_=fblk(3, 3))
            nc.scalar.dma_start(out=F4[2][:, 3 * 1024:4 * 1024], in_=fblk(2, 3))
            for g in range(2, 8):
                nc.gpsimd.memset(Vng[g], 1.0)

            with tc.tile_pool(name="psA", bufs=1, space="PSUM") as psA:
                # ---- PE warmup: open the HAM clock-gate while DMA streams ----
                wps = psA.tile([128, 1024], f32, name="warm_ps", tag="qkv", bufs=2)

                def warm(n):
                    # density filler: keeps the HAM duty-cycle ramp fed while
                    # qkv groups wait on fmap stripe DMAs
                    for _ in range(n):
                        nc.tensor.matmul(wps[:, 0:512], warm_sb[:, 0:128],
                                         warm_sb, start=True, stop=True)

                # ---- phase A: qkv projection + bias + V build ----
                def qkv_group(dst, col0, t, eng):
                    # h-major so the h0 columns (whose fmap halves land first)
                    # are not gated on the h1 stripe DMAs
                    gps = psA.tile([128, 1024], f32, name="qkv_ps", tag="qkv", bufs=2)
                    for h in range(2):
                        for c in range(4):
                            nc.tensor.matmul(
                                gps[:, h * 512:(h + 1) * 512],
                                W4[c][:, col0:col0 + 128],
                                F4[c][:, t * 1024 + h * 512: t * 1024 + (h + 1) * 512],
                                start=(c == 0), stop=(c == 3))
                    if eng == "act":
                        nc.scalar.copy(dst, gps)
                    else:
                        nc.vector.tensor_copy(dst, gps)

                qt_g = QT.rearrange("d (i w) -> d w i", w=64)
                bt_w = BT[64:128, :].rearrange("p (i w) -> p i w", i=32, w=64)

                def bias_bh(blk):
                    # per-band rel-logit matmuls (window shift baked into the
                    # stationary operand) writing disjoint PSUM bands
                    bh_ps = psA.tile([64, 512], f32, name="bh_ps", tag="bias", bufs=2)
                    for r in range(8):
                        rr = blk * 8 + r
                        nc.tensor.matmul(
                            bh_ps[:, r * 64:(r + 1) * 64],
                            relh_sb[:, 31 - rr:95 - rr],
                            QT[:, rr * 64:(rr + 1) * 64],
                            start=True, stop=True)
                    nc.vector.tensor_copy(
                        BT[0:64, blk * 512:(blk + 1) * 512], bh_ps)

                def bias_bw(blk):
                    bw_ps = psA.tile([64, 512], f32, name="bw_ps", tag="bias", bufs=2)
                    for w in range(16):
                        ww = blk * 16 + w
                        nc.tensor.matmul(
                            bw_ps[:, w * 32:(w + 1) * 32],
                            relw_sb[:, 63 - ww:127 - ww],
                            qt_g[:, ww, :],
                            start=True, stop=True)
                    nc.vector.tensor_copy(
                        bt_w[:, :, blk * 16:(blk + 1) * 16],
                        bw_ps.rearrange("p (w i) -> p i w", w=16, i=32))

                def vn_group(g, eng, pool, tag, bufs):
                    # V chunks [key, d] built directly: fmap chunk stationary,
                    # W_v moving -- no PE transposes needed
                    vps = pool.tile([128, 512], f32, name="vn_ps", tag=tag, bufs=bufs)
                    for j in range(4):
                        kc = 4 * g + j
                        for c in range(4):
                            nc.tensor.matmul(
                                vps[:, j * 128:(j + 1) * 128],
                                F4[c][:, kc * 128:(kc + 1) * 128],
                                W4[c][:, 256:384],
                                start=(c == 0), stop=(c == 3))
                    dst = Vng[g].rearrange("p (j c) -> p j c", c=129)[:, :, 0:128]
                    src = vps.rearrange("p (j c) -> p j c", c=128)
                    if eng == "act":
                        nc.scalar.copy(dst, src)
                    else:
                        nc.vector.tensor_copy(dst, src)

                # t0-dependent work first (fmap t1-3 still in flight), bias
                # bh/bw split so each evac drains behind independent PE work
                warm(12)
                qkv_group(QT[:, 0:1024], 0, 0, "dve")
                warm(2)
                qkv_group(KTt[0], 128, 0, "act")
                warm(2)
                vn_group(0, "act", psA, "vn", 2)
                vn_group(1, "dve", psA, "vn", 2)
                qkv_group(QT[:, 1024:2048], 0, 1, "dve")
                bias_bh(0)
                bias_bh(1)
                bias_bw(0)
                bias_bw(1)
                bias_bh(2)
                bias_bh(3)
                bias_bw(2)
                bias_bw(3)

            # ---- phase C: attention main loop ----
            with tc.tile_pool(name="ps", bufs=1, space="PSUM") as ps:
                # outF: 8 q-blocks of [128, 129] at 256-col stride (each block
                # stays inside a 1KB half-bank); col 128 of each = rowsum
                prev = None        # (outF, expT, kc)
                norm_pending = None

                def outF_zero_init(outFa, outFb):
                    # two accumulation groups share each PSUM bank (2 blocks
                    # per 2KB bank); a start=True mid-stream clears the whole
                    # bank's has-written bits and drops the sibling group's
                    # first chunk.  Zero-init every block with a dummy
                    # start=True matmul (zeros stationary) up front, then
                    # accumulate with start=False only.
                    for b in range(8):
                        oF = outFa if b < 4 else outFb
                        nc.tensor.matmul(
                            oF[:, (b % 4) * 256:(b % 4) * 256 + 129],
                            warm_sb[:, 0:128], Vng[0][:, 0:129],
                            start=True, stop=False, skip_group_check=True)

                def outT_mms(outFa, outFb, expT, kc, blks=range(8)):
                    vsl = Vng[kc // 4][:, (kc % 4) * 129:((kc % 4) + 1) * 129]
                    for b in blks:
                        oF = outFa if b < 4 else outFb
                        nc.tensor.matmul(
                            oF[:, (b % 4) * 256:(b % 4) * 256 + 129],
                            expT[:, b * 128:(b + 1) * 128],
                            vsl,
                            start=False, stop=(kc == 31), skip_group_check=True)

                def normalize_half(qb, oF, hh):
                    rcols = work.tile([128, 4], f32, name="rcols", tag="rc", bufs=4)
                    rs_view = oF.rearrange("p (b c) -> p b c", c=256)[:, :, 128:129]
                    nc.vector.reciprocal_approx_fast(
                        out=rcols.rearrange("p (b o) -> p b o", o=1), in_=rs_view)
                    out_sb = work.tile([128, 512], fp16, name="out_sb", tag="osb", bufs=2)
                    for b in range(4):
                        nc.vector.tensor_scalar_mul(
                            out_sb[:, b * 128:(b + 1) * 128],
                            oF[:, b * 256:b * 256 + 128],
                            rcols[:, b:b + 1])
                    nc.sync.dma_start(
                        out=out_p[:, qb * QB + hh * 512:qb * QB + (hh + 1) * 512],
                        in_=out_sb)

                def normalize(qb, outFa, outFb):
                    normalize_half(qb, outFa, 0)
                    normalize_half(qb, outFb, 1)

                def qkv_group2(dst, col0, t):
                    # main-pool variant of qkv_group (psA is closed by now)
                    gps = ps.tile([128, 1024], f32, name="qkv_ps2", tag="sim", bufs=2)
                    for h in range(2):
                        for c in range(4):
                            nc.tensor.matmul(
                                gps[:, h * 512:(h + 1) * 512],
                                W4[c][:, col0:col0 + 128],
                                F4[c][:, t * 1024 + h * 512: t * 1024 + (h + 1) * 512],
                                start=(c == 0), stop=(c == 3))
                    nc.vector.tensor_copy(dst, gps)

                # deferred phase-A work, injected between qb0 chunks: each
                # filler's PE time also covers the previous chunk's exp
                # latency, so they ride along at near-zero pipeline cost
                fillers = {
                    2: lambda: qkv_group2(KTt[1], 128, 1),
                    4: lambda: vn_group(2, "dve", ps, "sim", 2),
                    6: lambda: vn_group(3, "dve", ps, "sim", 2),
                    8: lambda: qkv_group2(KTt[2], 128, 2),
                    10: lambda: vn_group(4, "dve", ps, "sim", 2),
                    12: lambda: vn_group(5, "dve", ps, "sim", 2),
                    14: lambda: qkv_group2(KTt[3], 128, 3),
                    16: lambda: vn_group(6, "dve", ps, "sim", 2),
                    18: lambda: vn_group(7, "dve", ps, "sim", 2),
                }

                for qb in range(2):
                    q0 = qb * QB
                    outFa = ps.tile([128, 1024], f32, name="outFa", tag="outa", bufs=1)
                    outFb = ps.tile([128, 1024], f32, name="outFb", tag="outb", bufs=1)
                    outF_zero_init(outFa, outFb)
                    for kc in range(32):
                        sim = ps.tile([128, QB], f32, name="sim", tag="sim", bufs=2)
                        for h in range(2):
                            sl = slice(q0 + h * 512, q0 + (h + 1) * 512)
                            po = sim[:, h * 512:(h + 1) * 512]
                            nc.tensor.matmul(
                                po, KTt[kc // 8][:, (kc % 8) * 128:(kc % 8 + 1) * 128],
                                QT[:, sl],
                                start=True, stop=False)
                            nc.tensor.matmul(
                                po, ind_sb[:, kc * 128:(kc + 1) * 128], BT[:, sl],
                                start=False, stop=True)
                        expT = work.tile([128, QB], fp16, name="expT", tag="exp", bufs=6)
                        nc.scalar.activation(expT, sim, EXPF,
                                             bias=bias4_sb[:, 0:1], scale=SCALE)
                        # software-pipeline: emit outT for the PREVIOUS chunk so
                        # the in-order PE queue never parks on this chunk's exp
                        if prev is not None:
                            outT_mms(*prev)
                            if norm_pending is not None:
                                normalize(*norm_pending)
                                norm_pending = None
                        if qb == 0 and kc in fillers:
                            fillers[kc]()
                        prev = (outFa, outFb, expT, kc)
                    norm_pending = (qb, outFa, outFb)
                # final flush: A-blocks stop first so their normalize overlaps
                # the B-block output matmuls
                outFa, outFb, expT, kc = prev
                outT_mms(outFa, outFb, expT, kc, blks=range(4))
                normalize_half(1, outFa, 0)
                outT_mms(outFa, outFb, expT, kc, blks=range(4, 8))
                normalize_half(1, outFb, 1)

    nc.finalize()
    return nc


def _prep_core_inputs(fmap, w_qkv, rel_height, rel_width, core):
    bf = ml_dtypes.bfloat16
    h, half = core // 2, core % 2
    q0 = half * NQ
    perm = (np.arange(L) + q0) % L
    fmap_flat = fmap.reshape(C, L)
    fmap_core = np.ascontiguousarray(fmap_flat[:, perm]).astype(bf)
    rows = np.r_[h * 128:(h + 1) * 128,
                 512 + h * 128:512 + (h + 1) * 128,
                 1024 + h * 128:1024 + (h + 1) * 128]
    wt = np.ascontiguousarray(w_qkv[rows].T).astype(bf)
    relhT = rel_height.T  # (128, 127)
    a = 32 * (1 - half)
    relh_slab = np.zeros((128, 96), np.float32)
    relh_slab[:, :95] = relhT[:, a:a + 95]
    relw = np.ascontiguousarray(rel_width.T).astype(bf)
    j = np.arange(L)
    ind = np.zeros((128, L), np.float32)
    ind[(j // 64 + 32 * half) % 64, j] = 1.0
    ind[64 + (j % 64), j] = 1.0
    fmap_blocks = np.ascontiguousarray(
        fmap_core.reshape(4, 128, 4, 1024).transpose(0, 2, 1, 3).reshape(16 * 128, 1024))
    ind_blocks = np.ascontiguousarray(
        ind.reshape(128, 4, 1024).transpose(1, 0, 2).reshape(4 * 128, 1024))

    return {
        "fmapc": fmap_blocks,
        "wt": wt,
        "relh": relh_slab.astype(bf),
        "relw": relw,
        "ind": ind_blocks.astype(bf),
        "bias4": np.full((128, 1), -4.0, np.float32),
    }


def _install_trace_hook():
    """Register the axon NTFF profiling hook (missing antenv.axon_hooks shim)
    and neuter the artifact upload so tracing works in this sandbox."""
    import sys
    import types
    import concourse.bass_utils as bu
    bu.upload_artifacts = lambda d: d
    try:
        from antenv import axon_hooks  # noqa: F401
        return
    except ImportError:
        pass
    import antenv
    mod = types.ModuleType("antenv.axon_hooks")
    mod._hook = None
    def set_axon_ntff_profile_hook(h):
        mod._hook = h
    def get_axon_ntff_profile_hook():
        return mod._hook
    mod.set_axon_ntff_profile_hook = set_axon_ntff_profile_hook
    mod.get_axon_ntff_profile_hook = get_axon_ntff_profile_hook
    sys.modules["antenv.axon_hooks"] = mod
    antenv.axon_hooks = mod
    try:
        from trn_agent_boot.trn_boot import _ntff_profile_via_ctypes
        h = _ntff_profile_via_ctypes("/opt/axon/libaxon_pjrt.so")
        if h is not None:
            mod._hook = h
    except Exception as e:
        print(f"trace hook install failed: {e}")


def kernel(fmap, w_qkv, rel_height, rel_width, _trace=False):
    global _GRAPH
    from concourse.bass_utils import run_bass_kernel_spmd

    fmap = np.asarray(fmap, dtype=np.float32)
    w_qkv = np.asarray(w_qkv, dtype=np.float32)
    rel_height = np.asarray(rel_height, dtype=np.float32)
    rel_width = np.asarray(rel_width, dtype=np.float32)

    if _GRAPH is None:
        _GRAPH = _build_graph()
    nc = _GRAPH

    in_maps = [_prep_core_inputs(fmap, w_qkv, rel_height, rel_width, c)
               for c in range(NCORES)]
    kw = {}
    if _trace:
        _install_trace_hook()
        import os
        os.makedirs("/tmp/ktrace", exist_ok=True)
        import tempfile
        kw = dict(tmpdir=tempfile.mkdtemp(dir="/tmp/ktrace"))
    res = None
    last_err = None
    for attempt in range(3):
        try:
            res = run_bass_kernel_spmd(nc, in_maps, core_ids=list(range(NCORES)),
                                       trace=_trace, **kw)
            break
        except Exception as e:  # transient PJRT/tunnel hiccups
            last_err = e
    if res is None:
        raise last_err
    out_full = np.zeros((C, L), np.float32)
    for c in range(NCORES):
        h, half = c // 2, c % 2
        arr = np.asarray(res.results[c]["out"], dtype=np.float32)  # [128, 2048]
        # arr[p, qb*1024 + b*128 + d] = out[d, q = qb*1024 + b*128 + p]
        qd = arr.reshape(128, 2, 8, 128).transpose(1, 2, 0, 3).reshape(NQ, 128)
        out_full[h * 128:(h + 1) * 128, half * NQ:(half + 1) * NQ] = qd.T
    if _trace:
        kernel._last_exec_time_ns = res.exec_time_ns
        kernel._last_profile = res.profile_json
    return out_full.reshape(1, C, H, W)


# revision 21
# speedup vs baseline: 1.0177x; 1.0177x over previous
"""Trainium2 Bass kernel for BotNet-style attention (4 heads, 64x64 map,
dh=128, decomposed 2D relative position bias).

Sharding: 8 cores = 4 heads x 2 query-halves. Each core computes its head's
q/k/v from the full fmap, builds the rel-pos bias row tensors on chip, and
runs attention in "transposed sim" orientation (keys on partitions, queries
on free dim):

  simT[k, q] = K^T.T @ Q^T  (+ bias via indicator-matmul accumulation)
  expT = exp(SCALE * simT - 4)              (ACT, PSUM->SBUF fp16)

The output matmul is flipped (expT chunk as stationary, V as moving) so the
result lands as [q, d] blocks, and V carries an extra all-ones column so the
softmax denominator accumulates in the same PSUM tile for free:

  out[q, 0:128 | 128] = sum_k expT[k, q-blk]^T @ [V[k, :] | 1]

Normalization is then a per-partition reciprocal + tensor_scalar multiply on
DVE (no cross-partition reduce, no broadcast matmul, no accumulate chain).

V is built directly in [key, d] orientation via matmul(F_chunk, W_v) with the
fmap chunk as the stationary operand, so no PE transposes are needed.

The rel-pos bias decomposes per query q=(hq,wq), key k=(hk,wk) as
  bias = Rh[q, hk-hq+63] + Rw[q, wk-wq+63]
The row tensors BT are built from per-window matmuls writing disjoint PSUM
bands and fold into sim via one accumulating matmul against a 0/1 indicator
matrix per key chunk.

Per-core inputs are key-permuted (own query half first) so the SPMD graph is
identical across cores; all per-core differences live in the input data.
"""

import numpy as np
import ml_dtypes

C, H, W = 512, 64, 64
HEADS, DH = 4, 128
L = H * W           # 4096
NQ = L // 2         # 2048 queries per core
QB = 1024           # query block
SCALE = DH ** -0.5
NCORES = 8

_GRAPH = None


def _build_graph():
    from concourse import bacc
    import concourse.mybir as mybir
    import concourse.tile as tile

    f32 = mybir.dt.float32
    bf16 = mybir.dt.bfloat16
    fp16 = mybir.dt.float16
    EXPF = mybir.ActivationFunctionType.Exp

    nc = bacc.Bacc(None)

    fmap_p = nc.declare_dram_parameter("fmapc", [16 * 128, 1024], bf16, isOutput=False)
    wt_p = nc.declare_dram_parameter("wt", [C, 384], bf16, isOutput=False)
    relh_p = nc.declare_dram_parameter("relh", [128, 96], bf16, isOutput=False)
    relw_p = nc.declare_dram_parameter("relw", [128, 127], bf16, isOutput=False)
    ind_p = nc.declare_dram_parameter("ind", [4 * 128, 1024], bf16, isOutput=False)
    bias4_p = nc.declare_dram_parameter("bias4", [128, 1], f32, isOutput=False)
    out_p = nc.declare_dram_parameter("out", [128, NQ], fp16, isOutput=True)

    with tile.TileContext(nc) as tc:
        with tc.tile_pool(name="const", bufs=1) as cpool, \
             tc.tile_pool(name="big", bufs=1) as big, \
             tc.tile_pool(name="work", bufs=2) as work:

            # warm tile memset first in the gpsimd stream so PE warmup
            # matmuls can start right after the init barrier
            warm_sb = work.tile([128, 512], bf16, name="warm_sb", tag="warm")
            nc.gpsimd.memset(warm_sb, 0.0)

            relh_sb = cpool.tile([128, 96], bf16, name="relh_sb")
            relw_sb = cpool.tile([128, 127], bf16, name="relw_sb")
            ind_sb = cpool.tile([128, L], bf16, name="ind_sb")
            bias4_sb = cpool.tile([128, 1], f32, name="bias4_sb")

            F4 = [big.tile([128, L], bf16, name=f"F{c}") for c in range(4)]
            W4 = []
            for c in range(4):
                W4.append(big.tile([128, 384], bf16, name=f"W{c}"))

            # V with a ones column per 129-wide key-chunk slab: cols
            # [j*129, j*129+128) = V rows, col j*129+128 = 1.0.  One tile per
            # 4-chunk group so a consumer only waits on its own group's evac
            # (tile-granular deps) -- required for mid-loop V build fillers.
            Vng = [big.tile([128, 4 * 129], fp16, name=f"Vn{g}") for g in range(8)]
            nc.gpsimd.memset(Vng[0], 1.0)
            nc.gpsimd.memset(Vng[1], 1.0)

            QT = big.tile([128, NQ], bf16, name="QT")
            KTt = [big.tile([128, 1024], bf16, name=f"KT{t}") for t in range(4)]
            BT = big.tile([128, NQ], bf16, name="BT")

            # ---- DMA: weights first (unblock qkv matmuls), fmap t-major
            # striped across three engine queues, ind last ----
            def fblk(c, t):
                b = c * 4 + t
                return fmap_p[b * 128:(b + 1) * 128, :]

            nc.sync.dma_start(out=W4[0], in_=wt_p[0:128, :])
            nc.scalar.dma_start(out=W4[1], in_=wt_p[128:256, :])
            nc.scalar.dma_start(out=W4[2], in_=wt_p[256:384, :])
            nc.sync.dma_start(out=W4[3], in_=wt_p[384:512, :])
            # tiny consts ride the hw queues ahead of fmap (SWDGE is far too
            # slow for anything the bias phase waits on)
            nc.sync.dma_start(out=relh_sb, in_=relh_p[:, :])
            nc.scalar.dma_start(out=relw_sb, in_=relw_p[:, :])
            nc.sync.dma_start(out=bias4_sb, in_=bias4_p[:, :])
            # fmap: c0+c3 on sync, c1+c2 on scalar (only SP/ACT have hw DGE)
            for h in range(2):
                nc.sync.dma_start(out=F4[0][:, h * 512:(h + 1) * 512],
                                  in_=fblk(0, 0)[:, h * 512:(h + 1) * 512])
                nc.scalar.dma_start(out=F4[1][:, h * 512:(h + 1) * 512],
                                    in_=fblk(1, 0)[:, h * 512:(h + 1) * 512])
                nc.sync.dma_start(out=F4[3][:, h * 512:(h + 1) * 512],
                                  in_=fblk(3, 0)[:, h * 512:(h + 1) * 512])
                nc.scalar.dma_start(out=F4[2][:, h * 512:(h + 1) * 512],
                                    in_=fblk(2, 0)[:, h * 512:(h + 1) * 512])
            for t in range(1, 3):
                nc.sync.dma_start(out=F4[0][:, t * 1024:(t + 1) * 1024], in_=fblk(0, t))
                nc.scalar.dma_start(out=F4[1][:, t * 1024:(t + 1) * 1024], in_=fblk(1, t))
                nc.sync.dma_start(out=F4[3][:, t * 1024:(t + 1) * 1024], in_=fblk(3, t))
                nc.scalar.dma_start(out=F4[2][:, t * 1024:(t + 1) * 1024], in_=fblk(2, t))
            for k in range(4):
                (nc.sync if k % 2 == 0 else nc.scalar).dma_start(
                    out=ind_sb[:, k * 1024:(k + 1) * 1024],
                    in_=ind_p[k * 128:(k + 1) * 128, :])
            nc.sync.dma_start(out=F4[0][:, 3 * 1024:4 * 1024], in_=fblk(0, 3))
            nc.scalar.dma_start(out=F4[1][:, 3 * 1024:4 * 1024], in_=fblk(1, 3))
            nc.sync.dma_start(out=F4[3][:, 3 * 1024:4 * 1024], in_=fblk(3, 3))
            nc.scalar.dma_start(out=F4[2][:, 3 * 1024:4 * 1024], in_=fblk(2, 3))
            for g in range(2, 8):
                nc.gpsimd.memset(Vng[g], 1.0)

            with tc.tile_pool(name="psA", bufs=1, space="PSUM") as psA:
                # ---- PE warmup: open the HAM clock-gate while DMA streams ----
                wps = psA.tile([128, 1024], f32, name="warm_ps", tag="qkv", bufs=2)

                def warm(n):
                    # density filler: keeps the HAM duty-cycle ramp fed while
                    # qkv groups wait on fmap stripe DMAs
                    for _ in range(n):
                        nc.tensor.matmul(wps[:, 0:512], warm_sb[:, 0:128],
                                         warm_sb, start=True, stop=True)

                # ---- phase A: qkv projection + bias + V build ----
                def qkv_group(dst, col0, t, eng):
                    # h-major so the h0 columns (whose fmap halves land first)
                    # are not gated on the h1 stripe DMAs
                    gps = psA.tile([128, 1024], f32, name="qkv_ps", tag="qkv", bufs=2)
                    for h in range(2):
                        for c in range(4):
                            nc.tensor.matmul(
                                gps[:, h * 512:(h + 1) * 512],
                                W4[c][:, col0:col0 + 128],
                                F4[c][:, t * 1024 + h * 512: t * 1024 + (h + 1) * 512],
                                start=(c == 0), stop=(c == 3))
                    if eng == "act":
                        nc.scalar.copy(dst, gps)
                    else:
                        nc.vector.tensor_copy(dst, gps)

                qt_g = QT.rearrange("d (i w) -> d w i", w=64)
                bt_w = BT[64:128, :].rearrange("p (i w) -> p i w", i=32, w=64)

                def bias_bh(blk):
                    # per-band rel-logit matmuls (window shift baked into the
                    # stationary operand) writing disjoint PSUM bands
                    bh_ps = psA.tile([64, 512], f32, name="bh_ps", tag="bias", bufs=2)
                    for r in range(8):
                        rr = blk * 8 + r
                        nc.tensor.matmul(
                            bh_ps[:, r * 64:(r + 1) * 64],
                            relh_sb[:, 31 - rr:95 - rr],
                            QT[:, rr * 64:(rr + 1) * 64],
                            start=True, stop=True)
                    nc.vector.tensor_copy(
                        BT[0:64, blk * 512:(blk + 1) * 512], bh_ps)

                def bias_bw(blk):
                    bw_ps = psA.tile([64, 512], f32, name="bw_ps", tag="bias", bufs=2)
                    for w in range(16):
                        ww = blk * 16 + w
                        nc.tensor.matmul(
                            bw_ps[:, w * 32:(w + 1) * 32],
                            relw_sb[:, 63 - ww:127 - ww],
                            qt_g[:, ww, :],
                            start=True, stop=True)
                    nc.vector.tensor_copy(
                        bt_w[:, :, blk * 16:(blk + 1) * 16],
                        bw_ps.rearrange("p (w i) -> p i w", w=16, i=32))

                def vn_group(g, eng, pool, tag, bufs):
                    # V chunks [key, d] built directly: fmap chunk stationary,
                    # W_v moving -- no PE transposes needed
                    vps = pool.tile([128, 512], f32, name="vn_ps", tag=tag, bufs=bufs)
                    for j in range(4):
                        kc = 4 * g + j
                        for c in range(4):
                            nc.tensor.matmul(
                                vps[:, j * 128:(j + 1) * 128],
                                F4[c][:, kc * 128:(kc + 1) * 128],
                                W4[c][:, 256:384],
                                start=(c == 0), stop=(c == 3))
                    dst = Vng[g].rearrange("p (j c) -> p j c", c=129)[:, :, 0:128]
                    src = vps.rearrange("p (j c) -> p j c", c=128)
                    if eng == "act":
                        nc.scalar.copy(dst, src)
                    else:
                        nc.vector.tensor_copy(dst, src)

                # t0-dependent work first (fmap t1-3 still in flight), bias
                # bh/bw split so each evac drains behind independent PE work
                warm(12)
                qkv_group(QT[:, 0:1024], 0, 0, "dve")
                warm(2)
                qkv_group(KTt[0], 128, 0, "act")
                warm(2)
                qkv_group(QT[:, 1024:2048], 0, 1, "dve")
                vn_group(0, "act", psA, "vn", 2)
                vn_group(1, "dve", psA, "vn", 2)
                bias_bh(0)
                bias_bh(1)
                qkv_group(KTt[1], 128, 1, "act")
                bias_bw(0)
                bias_bw(1)
                bias_bh(2)
                bias_bh(3)
                vn_group(2, "dve", psA, "vn", 2)
                bias_bw(2)
                bias_bw(3)
                qkv_group(KTt[2], 128, 2, "act")
                vn_group(3, "dve", psA, "vn", 2)
                vn_group(4, "act", psA, "vn", 2)
                qkv_group(KTt[3], 128, 3, "act")
                vn_group(5, "dve", psA, "vn", 2)
                vn_group(6, "act", psA, "vn", 2)
                vn_group(7, "dve", psA, "vn", 2)

            # ---- phase C: attention main loop ----
            with tc.tile_pool(name="ps", bufs=1, space="PSUM") as ps:
                # outF: 8 q-blocks of [128, 129] at 256-col stride (each block
                # stays inside a 1KB half-bank); col 128 of each = rowsum
                prev = None        # (outF, expT, kc)
                norm_pending = None

                def outF_zero_init(outFa, outFb):
                    # two accumulation groups share each PSUM bank (2 blocks
                    # per 2KB bank); a start=True mid-stream clears the whole
                    # bank's has-written bits and drops the sibling group's
                    # first chunk.  Zero-init every block with a dummy
                    # start=True matmul (zeros stationary) up front, then
                    # accumulate with start=False only.
                    for b in range(8):
                        oF = outFa if b < 4 else outFb
                        nc.tensor.matmul(
                            oF[:, (b % 4) * 256:(b % 4) * 256 + 129],
                            warm_sb[:, 0:128], Vng[0][:, 0:129],
                            start=True, stop=False, skip_group_check=True)

                def outT_mms(outFa, outFb, expT, kc, blks=range(8)):
                    vsl = Vng[kc // 4][:, (kc % 4) * 129:((kc % 4) + 1) * 129]
                    for b in blks:
                        oF = outFa if b < 4 else outFb
                        nc.tensor.matmul(
                            oF[:, (b % 4) * 256:(b % 4) * 256 + 129],
                            expT[:, b * 128:(b + 1) * 128],
                            vsl,
                            start=False, stop=(kc == 31), skip_group_check=True)

                def normalize_half(qb, oF, hh):
                    rcols = work.tile([128, 4], f32, name="rcols", tag="rc", bufs=4)
                    rs_view = oF.rearrange("p (b c) -> p b c", c=256)[:, :, 128:129]
                    nc.vector.reciprocal_approx_fast(
                        out=rcols.rearrange("p (b o) -> p b o", o=1), in_=rs_view)
                    out_sb = work.tile([128, 512], fp16, name="out_sb", tag="osb", bufs=2)
                    for b in range(4):
                        nc.vector.tensor_scalar_mul(
                            out_sb[:, b * 128:(b + 1) * 128],
                            oF[:, b * 256:b * 256 + 128],
                            rcols[:, b:b + 1])
                    nc.sync.dma_start(
                        out=out_p[:, qb * QB + hh * 512:qb * QB + (hh + 1) * 512],
                        in_=out_sb)

                def normalize(qb, outFa, outFb):
                    normalize_half(qb, outFa, 0)
                    normalize_half(qb, outFb, 1)

                for qb in range(2):
                    q0 = qb * QB
                    outFa = ps.tile([128, 1024], f32, name="outFa", tag="outa", bufs=1)
                    outFb = ps.tile([128, 1024], f32, name="outFb", tag="outb", bufs=1)
                    outF_zero_init(outFa, outFb)
                    for kc in range(32):
                        sim = ps.tile([128, QB], f32, name="sim", tag="sim", bufs=2)
                        for h in range(2):
                            sl = slice(q0 + h * 512, q0 + (h + 1) * 512)
                            po = sim[:, h * 512:(h + 1) * 512]
                            nc.tensor.matmul(
                                po, KTt[kc // 8][:, (kc % 8) * 128:(kc % 8 + 1) * 128],
                                QT[:, sl],
                                start=True, stop=False)
                            nc.tensor.matmul(
                                po, ind_sb[:, kc * 128:(kc + 1) * 128], BT[:, sl],
                                start=False, stop=True)
                        expT = work.tile([128, QB], fp16, name="expT", tag="exp", bufs=6)
                        nc.scalar.activation(expT, sim, EXPF,
                                             bias=bias4_sb[:, 0:1], scale=SCALE)
                        # software-pipeline: emit outT for the PREVIOUS chunk so
                        # the in-order PE queue never parks on this chunk's exp
                        if prev is not None:
                            outT_mms(*prev)
                            if norm_pending is not None:
                                normalize(*norm_pending)
                                norm_pending = None
                        prev = (outFa, outFb, expT, kc)
                    norm_pending = (qb, outFa, outFb)
                # final flush: A-blocks stop first so their normalize overlaps
                # the B-block output matmuls
                outFa, outFb, expT, kc = prev
                outT_mms(outFa, outFb, expT, kc, blks=range(4))
                normalize_half(1, outFa, 0)
                outT_mms(outFa, outFb, expT, kc, blks=range(4, 8))
                normalize_half(1, outFb, 1)

    nc.finalize()
    return nc


def _prep_core_inputs(fmap, w_qkv, rel_height, rel_width, core):
    bf = ml_dtypes.bfloat16
    h, half = core // 2, core % 2
    q0 = half * NQ
    perm = (np.arange(L) + q0) % L
    fmap_flat = fmap.reshape(C, L)
    fmap_core = np.ascontiguousarray(fmap_flat[:, perm]).astype(bf)
    rows = np.r_[h * 128:(h + 1) * 128,
                 512 + h * 128:512 + (h + 1) * 128,
                 1024 + h * 128:1024 + (h + 1) * 128]
    wt = np.ascontiguousarray(w_qkv[rows].T).astype(bf)
    relhT = rel_height.T  # (128, 127)
    a = 32 * (1 - half)
    relh_slab = np.zeros((128, 96), np.float32)
    relh_slab[:, :95] = relhT[:, a:a + 95]
    relw = np.ascontiguousarray(rel_width.T).astype(bf)
    j = np.arange(L)
    ind = np.zeros((128, L), np.float32)
    ind[(j // 64 + 32 * half) % 64, j] = 1.0
    ind[64 + (j % 64), j] = 1.0
    fmap_blocks = np.ascontiguousarray(
        fmap_core.reshape(4, 128, 4, 1024).transpose(0, 2, 1, 3).reshape(16 * 128, 1024))
    ind_blocks = np.ascontiguousarray(
        ind.reshape(128, 4, 1024).transpose(1, 0, 2).reshape(4 * 128, 1024))

    return {
        "fmapc": fmap_blocks,
        "wt": wt,
        "relh": relh_slab.astype(bf),
        "relw": relw,
        "ind": ind_blocks.astype(bf),
        "bias4": np.full((128, 1), -4.0, np.float32),
    }


def _install_trace_hook():
    """Register the axon NTFF profiling hook (missing antenv.axon_hooks shim)
    and neuter the artifact upload so tracing works in this sandbox."""
    import sys
    import types
    import concourse.bass_utils as bu
    bu.upload_artifacts = lambda d: d
    try:
        from antenv import axon_hooks  # noqa: F401
        return
    except ImportError:
        pass
    import antenv
    mod = types.ModuleType("antenv.axon_hooks")
    mod._hook = None
    def set_axon_ntff_profile_hook(h):
        mod._hook = h
    def get_axon_ntff_profile_hook():
        return mod._hook
    mod.set_axon_ntff_profile_hook = set_axon_ntff_profile_hook
    mod.get_axon_ntff_profile_hook = get_axon_ntff_profile_hook
    sys.modules["antenv.axon_hooks"] = mod
    antenv.axon_hooks = mod
    try:
        from trn_agent_boot.trn_boot import _ntff_profile_via_ctypes
        h = _ntff_profile_via_ctypes("/opt/axon/libaxon_pjrt.so")
        if h is not None:
            mod._hook = h
    except Exception as e:
        print(f"trace hook install failed: {e}")


def kernel(fmap, w_qkv, rel_height, rel_width, _trace=False):
    global _GRAPH
    from concourse.bass_utils import run_bass_kernel_spmd

    fmap = np.asarray(fmap, dtype=np.float32)
    w_qkv = np.asarray(w_qkv, dtype=np.float32)
    rel_height = np.asarray(rel_height, dtype=np.float32)
    rel_width = np.asarray(rel_width, dtype=np.float32)

    if _GRAPH is None:
        _GRAPH = _build_graph()
    nc = _GRAPH

    in_maps = [_prep_core_inputs(fmap, w_qkv, rel_height, rel_width, c)
               for c in range(NCORES)]
    kw = {}
    if _trace:
        _install_trace_hook()
        import os
        os.makedirs("/tmp/ktrace", exist_ok=True)
        import tempfile
        kw = dict(tmpdir=tempfile.mkdtemp(dir="/tmp/ktrace"))
    res = None
    last_err = None
    for attempt in range(3):
        try:
            res = run_bass_kernel_spmd(nc, in_maps, core_ids=list(range(NCORES)),
                                       trace=_trace, **kw)
            break
        except Exception as e:  # transient PJRT/tunnel hiccups
            last_err = e
    if res is None:
        raise last_err
    out_full = np.zeros((C, L), np.float32)
    for c in range(NCORES):
        h, half = c // 2, c % 2
        arr = np.asarray(res.results[c]["out"], dtype=np.float32)  # [128, 2048]
        # arr[p, qb*1024 + b*128 + d] = out[d, q = qb*1024 + b*128 + p]
        qd = arr.reshape(128, 2, 8, 128).transpose(1, 2, 0, 3).reshape(NQ, 128)
        out_full[h * 128:(h + 1) * 128, half * NQ:(half + 1) * NQ] = qd.T
    if _trace:
        kernel._last_exec_time_ns = res.exec_time_ns
        kernel._last_profile = res.profile_json
    return out_full.reshape(1, C, H, W)
